# revision 54
# baseline (speedup 1.0000x reference)
"""Bass/Tile TRN2 kernel for the attention module:

    pre    = prev_hidden @ W1[:H] + b1                    [B, H]
    hidden = tanh(pre[:, None, :] + ann @ W1[H:])         [B, S, H]
    score  = hidden @ W2 (+ b2; softmax-invariant, drop)  [B, S]
    alpha  = softmax(score, axis=1)
    ctx    = alpha @ ann                                  [B, 1, A]

B=32, S=4096, A=H=512. Sharding: data-parallel over batch, 4 batches per
core on 8 cores. Single pass over S per batch with an unnormalized
online softmax (scores are bounded: |score| <= sum|W2|+|b2| ~ 11.4, so
exp never overflows in fp32 and no running-max is needed):

    w_s = exp(score_s);  Z = sum w_s;  ctx = (sum w_s * ann_s) / Z

Layouts: the s-dim matmul (ann @ W1a) contracts over the feature dim a,
so it needs ann with a on SBUF partitions (annT, fp8 e4m3); the context
matmul contracts over s, so it needs natural ann (annN, bf16). The
ann @ W1a matmul runs in fp8 DoubleRow mode (K=256 per MM, 0.5
cycles/row): W1a is host-prescaled x32 into e4m3's normal range and the
1/32 is folded into the tanh activation's scale. W1h (the tiny pre
matmul, on the startup critical path feeding every tanh's bias) is
likewise fp8 x32, rescaled in the PSUM->SBUF copy.

Schedule: s-chunks of 512 are processed in PAIRS so each tanh covers
1024 elements per partition (fewer Activation-engine calls). The score
is computed TRANSPOSED (s on partitions) by 1-column matmuls whose
stationary operand is the tanh tile and whose moving operand is the W2
vector, so exp directly yields softmax-weight columns in the layout the
ctx matmuls consume -- no weight transpose exists anywhere. The ctx
matmuls likewise make annN the stationary operand and stream one
w-column each (output free size 1), accumulating per-pair PSUM groups
that the otherwise-idle DVE folds into an SBUF accumulator (PSUM allows
only one open accumulation group per zero region, so kernel-lifetime
column groups are illegal). Z comes from a ones-vector contraction of
the weight columns plus a small selector matmul at the end. Within a
pair the in-order PE interleaves step2 (fp8 DR), the previous batch's
score groups, and the previous pair's ctx groups as filler, so it never
head-blocks on the Activation engine's tanh stream. Slots iterate
b-major (all four hc of one batch before the next batch) so at startup
the Activation engine streams batch 0's tanh groups while the other
batches' annT tiles are still arriving. annT DMAs are issued one pair
ahead of use; annN (first needed by the deferred ctx tail one pair
later) trails in the same queue.
"""

import os

import numpy as np
import ml_dtypes

B = 32
S = 4096
A = 512
H = 512
NCORES = 8
BL = B // NCORES  # 4 batches per core
SC = 512          # s-chunk per matmul moving operand
NSC = S // SC     # 8
NPAIR = NSC // 2  # chunk pairs per core

BF16 = ml_dtypes.bfloat16
F8 = ml_dtypes.float8_e4m3  # maps to mybir.dt.float8e4 (TRN fp8 e4m3)
W1A_SCALE = 32.0  # W1a entries ~U(+-1/32); prescale into e4m3's normal range

_BUILT = None       # (nc,) cache — Bass module is reusable across calls
LAST_RESULT = None  # last BassKernelResults, for test harness introspection


def _build_bass(loop_n=None):
    """Build the Bass module. loop_n wraps the main pair-loop in a For_i
    executed loop_n times — a timing amplifier (outputs then meaningless);
    loop_n=None builds the real single-pass kernel."""
    from contextlib import ExitStack, nullcontext

    import concourse.bass as bass
    import concourse.tile as tile
    from concourse import bacc, mybir
    from concourse.masks import make_identity

    bf16 = mybir.dt.bfloat16
    f8 = mybir.dt.float8e4
    f32 = mybir.dt.float32

    nc = bacc.Bacc()

    annT_d = nc.dram_tensor("annT", [BL, A, S], f8, kind="ExternalInput")
    annN_d = nc.dram_tensor("annN", [BL, S, A], bf16, kind="ExternalInput")
    w1a_d = nc.dram_tensor("w1a", [A, H], f8, kind="ExternalInput")
    w1h_d = nc.dram_tensor("w1h", [H, H], f8, kind="ExternalInput")
    b1_d = nc.dram_tensor("b1", [1, H], bf16, kind="ExternalInput")
    w2_d = nc.dram_tensor("w2", [H, 1], bf16, kind="ExternalInput")
    # sel[k, b] = 1 if k % BL == b: partition-group selector for the Z
    # reduction (sums the per-(sub,st) column partials of batch b)
    sel_d = nc.dram_tensor("sel", [32, BL], f32, kind="ExternalInput")
    pvt_d = nc.dram_tensor("pvt", [H, BL], bf16, kind="ExternalInput")
    out_d = nc.dram_tensor("out", [BL, A], f32, kind="ExternalOutput")

    with tile.TileContext(nc) as tc, ExitStack() as ctx:
        singles = ctx.enter_context(tc.tile_pool(name="singles", bufs=1))
        annt_pool = ctx.enter_context(tc.tile_pool(name="annt", bufs=2))
        annn_pool = ctx.enter_context(tc.tile_pool(name="annn", bufs=2))
        th_pool = ctx.enter_context(tc.tile_pool(name="thp", bufs=2))
        w_pool = ctx.enter_context(tc.tile_pool(name="wp", bufs=2))
        psum_th = ctx.enter_context(
            tc.tile_pool(name="psumth", bufs=1, space="PSUM")
        )
        psum_sc = ctx.enter_context(
            tc.tile_pool(name="psumsc", bufs=1, space="PSUM")
        )
        psum_z = ctx.enter_context(
            tc.tile_pool(name="psumz", bufs=1, space="PSUM")
        )
        psum1 = ctx.enter_context(
            tc.tile_pool(name="psum1", bufs=1, space="PSUM")
        )

        # ---- weights / constants into SBUF ----
        # DMA queue order matters: w1h/pvt/b1 first so the pre matmul (the
        # Activation engine's gating dependency for every tanh) lands as
        # early as possible; then w1a + the first pair's annT for the PE's
        # first step2 slots.
        w1h_sb = singles.tile([128, 4, H], f8)  # (hin%128, hin//128, h)
        nc.sync.dma_start(
            out=w1h_sb, in_=w1h_d[:, :].rearrange("(kc p) h -> p kc h", p=128)
        )
        pvt_sb = singles.tile([128, 4, BL], bf16)  # (hin%128, hin//128, b)
        nc.sync.dma_start(
            out=pvt_sb, in_=pvt_d[:, :].rearrange("(kc p) b -> p kc b", p=128)
        )
        b1_sb = singles.tile([1, H], bf16)
        nc.sync.dma_start(out=b1_sb, in_=b1_d[:, :])

        w1a_sb = singles.tile([128, 4, H], f8)  # (a%128, a//128, h)
        nc.sync.dma_start(
            out=w1a_sb, in_=w1a_d[:, :].rearrange("(ac p) h -> p ac h", p=128)
        )

        at_tiles = {}  # (sp, sub, b) -> tile, DMA-issued one pair ahead

        def issue_at(sp, bs):
            for b in bs:
                for sub in range(2):
                    sc = 2 * sp + sub
                    t = annt_pool.tile([128, 4, SC], f8, tag=f"at{sub}{b}")
                    nc.sync.dma_start(
                        out=t,
                        in_=annT_d[b, :, sc * SC:(sc + 1) * SC].rearrange(
                            "(ac p) s -> p ac s", p=128
                        ),
                    )
                    at_tiles[(sp, sub, b)] = t

        issue_at(0, (0, 1))
        issue_at(0, (2, 3))

        w2_sb = singles.tile([128, 4, 1], bf16)  # (h%128, h//128, 1)
        nc.sync.dma_start(
            out=w2_sb, in_=w2_d[:, :].rearrange("(hc p) r -> p hc r", p=128)
        )
        sel_sb = singles.tile([32, BL], f32)
        nc.sync.dma_start(out=sel_sb, in_=sel_d[:, :])

        ones_sb = singles.tile([1, BL], bf16)
        nc.vector.memset(ones_sb, 1.0)
        onesc_sb = singles.tile([128, 1], bf16)
        nc.vector.memset(onesc_sb, 1.0)
        onesr_sb = singles.tile([1, 128], f32)
        nc.vector.memset(onesr_sb, 1.0)
        ones32_sb = singles.tile([32, 1], f32)
        nc.vector.memset(ones32_sb, 1.0)
        identf = singles.tile([128, 128], f32)
        make_identity(nc, identf)

        pre_sb = singles.tile([128, 4, BL], f32)

        def emit_pre():
            # pre2T[h, b] = (prev @ W1h).T + b1 broadcast.
            pre_ps = psum_sc.tile([128, 32], f32, tag="score")
            for hc in range(4):
                for kc in range(4):
                    nc.tensor.matmul(
                        pre_ps[:, hc * BL:(hc + 1) * BL],
                        lhsT=w1h_sb[:, kc, hc * 128:(hc + 1) * 128],
                        rhs=pvt_sb[:, kc, :],
                        start=(kc == 0),
                        stop=False,
                    )
                # b1 contribution: rank-1 with ones row (K=1)
                nc.tensor.matmul(
                    pre_ps[:, hc * BL:(hc + 1) * BL],
                    lhsT=b1_sb[:, hc * 128:(hc + 1) * 128],
                    rhs=ones_sb[:, :],
                    start=False,
                    stop=True,
                )
            nc.scalar.activation(
                out=pre_sb,
                in_=pre_ps[:, 0:4 * BL].rearrange(
                    "p (hc b) -> p hc b", b=BL
                ),
                func=mybir.ActivationFunctionType.Identity,
                scale=1.0 / W1A_SCALE,
            )

        ctx_acc = singles.tile([128, BL, 4], f32)  # (a%128, b, ac)
        nc.vector.memset(ctx_acc, 0.0)
        zp_acc = singles.tile([32, 1], f32)  # per-(sub,st,b) Z partials
        nc.vector.memset(zp_acc, 0.0)

        outer = (
            tc.For_i(0, loop_n, 1) if loop_n is not None else nullcontext()
        )
        with outer:
            _main_body(
                nc, tc, mybir,
                annN_d, w1a_sb, w2_sb, pre_sb,
                at_tiles, issue_at, emit_pre,
                annn_pool, th_pool, w_pool, psum_th, psum_sc, psum_z,
                psum1, onesc_sb, ctx_acc, zp_acc,
                amplified=loop_n is not None,
            )

        # ---- normalize and store ----
        # Z per batch: group-sum the [32,1] column partials via the selector
        # matmul, reciprocal, transpose to a row, broadcast over partitions,
        # then scale ctx and store.
        zpsel_sb = singles.tile([32, BL], f32)
        nc.vector.tensor_scalar_mul(zpsel_sb, sel_sb[:, :], zp_acc[:, 0:1])
        zbt_ps = psum_sc.tile([1, BL], f32, tag="score")
        nc.tensor.matmul(
            zbt_ps[:, :], lhsT=ones32_sb[:, :], rhs=zpsel_sb[:, :],
            start=True, stop=True,
        )
        zrt_sb = singles.tile([1, BL], f32)
        nc.vector.reciprocal(out=zrt_sb, in_=zbt_ps)
        bc_ps = psum_sc.tile([128, BL], f32, tag="score")
        nc.tensor.matmul(
            bc_ps[:, :], lhsT=onesr_sb[:, :], rhs=zrt_sb[:, :],
            start=True, stop=True,
        )
        out_sb = singles.tile([128, BL, 4], f32)
        for b in range(BL):
            nc.vector.tensor_scalar_mul(
                out_sb[:, b, :], ctx_acc[:, b, :], bc_ps[:, b:b + 1]
            )
        outt_ps = psum_sc.tile([4 * BL, 128], f32, tag="score")
        nc.tensor.transpose(
            outt_ps[:, :],
            out_sb[:, :, :].rearrange("p b ac -> p (b ac)"),
            identf[:, :],
        )
        outt_sb = singles.tile([4 * BL, 128], f32)
        nc.vector.tensor_copy(out=outt_sb, in_=outt_ps[:, :])
        nc.sync.dma_start(
            out=out_d[:, :].rearrange("b (ac p) -> (b ac) p", p=128),
            in_=outt_sb,
        )

    nc.finalize()
    return nc


def _main_body(
    nc, tc, mybir,
    annN_d, w1a_sb, w2_sb, pre_sb,
    at_tiles, issue_at, emit_pre,
    annn_pool, th_pool, w_pool, psum_th, psum_sc, psum_z,
    psum1, onesc_sb, ctx_acc, zp_acc,
    amplified=False,
):
    bf16 = mybir.dt.bfloat16
    f8 = mybir.dt.float8e4
    f32 = mybir.dt.float32
    Tanh = mybir.ActivationFunctionType.Tanh
    Exp = mybir.ActivationFunctionType.Exp
    DR = mybir.MatmulPerfMode.DoubleRow

    # Score is computed TRANSPOSED (s on partitions): scoreT[s, col] with
    # col = sub*16 + st*4 + b, via tiny 1-column matmuls whose STATIONARY
    # operand is the th tile (lhsT) and whose moving operand is the W2
    # vector. exp then produces the softmax weights already in the layout
    # the ctx matmuls need as moving columns, so no w transpose exists at
    # all. The ctx matmuls likewise make annN the stationary operand and
    # stream one w column (out free size 1). The ctx MMs of pair sp-1 are
    # deferred into iteration sp and spread through its slots as filler.
    emit_pre()
    pend = None
    for sp in range(NPAIR + 1):
        if sp < NPAIR:
            if amplified:
                if sp > 0:
                    issue_at(sp, range(BL))  # no prefetch under For_i
            elif sp + 1 < NPAIR:
                issue_at(sp + 1, range(BL))  # prefetch next pair's annT
            an_tiles = []  # an_tiles[b][sub]
            for b in range(BL):
                subs = []
                for sub in range(2):
                    sc = 2 * sp + sub
                    t = annn_pool.tile([128, 4, A], bf16, tag=f"an{sub}{b}")
                    nc.sync.dma_start(
                        out=t,
                        in_=annN_d[b, sc * SC:(sc + 1) * SC, :].rearrange(
                            "(sb p) a -> p sb a", p=128
                        ),
                    )
                    subs.append(t)
                an_tiles.append(subs)

            th_tiles = [
                th_pool.tile(
                    [128, 4, 2, SC], bf16, tag=f"th{b}", name=f"th{b}"
                )
                for b in range(BL)
            ]
            score_ps = psum_sc.tile([128, 32], f32, tag="score")

        if pend is not None:
            p_w, p_an, p_sp = pend
            # one job per (ac, b) ctx column: its 8 st MMs are emitted
            # CONSECUTIVELY as a complete per-pair accumulation group (PSUM
            # allows only one open group per zero region at a time); the
            # per-pair partial is then folded into ctx_acc by the DVE
            ctx_ps = psum1.tile([128, BL, 4], f32, tag="ctx")
            ctx_jobs = [(ac, b) for ac in range(4) for b in range(BL)]
        else:
            ctx_jobs = []

        def emit_zp():
            # Z partials: column-sums of the previous pair's softmax
            # weights (ones-vector contraction over the s partitions),
            # then folded into zp_acc by the DVE
            zpp = psum_z.tile([32, 1], f32, tag="zpp")
            nc.tensor.matmul(
                zpp[:, :],
                lhsT=p_w[:, :],
                rhs=onesc_sb[:, :],
                start=True,
                stop=True,
            )
            nc.vector.tensor_tensor(
                out=zp_acc[:, :], in0=zp_acc[:, :], in1=zpp[:, :],
                op=mybir.AluOpType.add,
            )

        def emit_ctx(n):
            for _ in range(n):
                if not ctx_jobs:
                    return
                ac, b = ctx_jobs.pop(0)
                for st in range(8):
                    col = (st // 4) * 16 + (st % 4) * BL + b
                    nc.tensor.matmul(
                        ctx_ps[:, b, ac:ac + 1],
                        lhsT=p_an[b][st // 4][
                            :, st % 4, ac * 128:(ac + 1) * 128
                        ],
                        rhs=p_w[:, col:col + 1],
                        start=(st == 0),
                        stop=(st == 7),
                    )

        def fold_ctx():
            nc.vector.tensor_tensor(
                out=ctx_acc[:, :, :], in0=ctx_acc[:, :, :],
                in1=ctx_ps[:, :, :],
                op=mybir.AluOpType.add,
            )

        if sp < NPAIR:

            def emit_score(b):
                # scoreT[s, col] = sum_hc th[:, s].T @ w2[:, hc]; th is the
                # stationary operand, out free size is 1. Each column's 4
                # MMs are consecutive (complete group before the next).
                for sub in range(2):
                    for st in range(4):
                        col = sub * 16 + st * BL + b
                        for hc in range(4):
                            nc.tensor.matmul(
                                score_ps[:, col:col + 1],
                                lhsT=th_tiles[b][
                                    :, hc, sub, st * 128:(st + 1) * 128
                                ],
                                rhs=w2_sb[:, hc, :],
                                start=(hc == 0),
                                stop=(hc == 3),
                            )

            slot = 0
            for b in range(BL):
                for hc in range(4):
                    thp = psum_th.tile([128, 2, SC], f32, tag=f"thp{hc % 2}")
                    for sub in range(2):
                        for kh in range(2):
                            nc.tensor.matmul(
                                thp[:, sub, :],
                                lhsT=w1a_sb[
                                    :, 2 * kh:2 * kh + 2,
                                    hc * 128:(hc + 1) * 128,
                                ],
                                rhs=at_tiles[(sp, sub, b)][
                                    :, 2 * kh:2 * kh + 2, :
                                ],
                                start=(kh == 0),
                                stop=(kh == 1),
                                perf_mode=DR,
                            )
                    nc.scalar.activation(
                        out=th_tiles[b][:, hc, :, :],
                        in_=thp[:, :, :],
                        func=Tanh,
                        bias=pre_sb[:, hc, b:b + 1],
                        scale=1.0 / W1A_SCALE,
                    )
                    if hc == 0 and b > 0:
                        emit_score(b - 1)
                    if pend is not None and slot == 0:
                        emit_zp()
                    if slot >= 2:
                        emit_ctx(2)
                    slot += 1
            emit_score(BL - 1)
            emit_ctx(len(ctx_jobs))
            if pend is not None:
                fold_ctx()

            # one exp over all 32 scoreT columns -> softmax weights with s
            # on partitions, directly consumable by the ctx matmuls
            w_sb = w_pool.tile([128, 32], bf16, tag="w")
            nc.scalar.activation(
                out=w_sb, in_=score_ps[:, :], func=Exp,
            )
            pend = (w_sb, an_tiles, sp)
        else:
            # drain: Z partials + remaining ctx MMs of the last pair
            emit_zp()
            emit_ctx(len(ctx_jobs))
            fold_ctx()
            pend = None


def _make_in_maps(prev_hidden_state, annotations, W1, b1, W2):
    prev_hidden_state = np.asarray(prev_hidden_state, dtype=np.float32)
    annotations = np.asarray(annotations, dtype=np.float32)
    W1 = np.asarray(W1, dtype=np.float32)
    b1 = np.asarray(b1, dtype=np.float32)
    W2 = np.asarray(W2, dtype=np.float32)

    annN = annotations.astype(BF16)
    annT = np.ascontiguousarray(annotations.transpose(0, 2, 1)).astype(F8)
    w1h = np.ascontiguousarray(W1[:H] * W1A_SCALE).astype(F8)
    w1a = np.ascontiguousarray(W1[H:] * W1A_SCALE).astype(F8)
    b1r = (b1 * W1A_SCALE).reshape(1, H).astype(BF16)
    w2c = np.ascontiguousarray(W2.reshape(H, 1)).astype(BF16)
    sel = np.zeros((32, BL), dtype=np.float32)
    for k in range(32):
        sel[k, k % BL] = 1.0
    pvt = np.ascontiguousarray(prev_hidden_state.T).astype(BF16)  # [H, B]

    in_maps = []
    for c in range(NCORES):
        sl = slice(c * BL, (c + 1) * BL)
        in_maps.append(
            {
                "annT": np.ascontiguousarray(annT[sl]),
                "annN": np.ascontiguousarray(annN[sl]),
                "w1a": w1a,
                "w1h": w1h,
                "b1": b1r,
                "w2": w2c,
                "sel": sel,
                "pvt": np.ascontiguousarray(pvt[:, sl]),
            }
        )
    return in_maps


def kernel(prev_hidden_state, annotations, W1, b1, W2, b2, **_unused):
    global _BUILT, LAST_RESULT
    from concourse import bass_utils

    # b2 shifts every score equally; softmax is shift-invariant -> ignored.
    in_maps = _make_in_maps(prev_hidden_state, annotations, W1, b1, W2)

    if _BUILT is None:
        _BUILT = _build_bass()
    nc = _BUILT

    trace = bool(int(os.environ.get("KERNEL_TRACE", "0")))
    if not trace:
        # the NTFF trace path needs antenv.axon_hooks, absent in this
        # client -- make sure an ambient BASS_TRACE can't select it
        os.environ.setdefault("BASS_NEVER_TRACE", "1")
    res = bass_utils.run_bass_kernel_spmd(
        nc, in_maps, core_ids=list(range(NCORES)), trace=trace
    )
    LAST_RESULT = res
    out = np.concatenate([r["out"] for r in res.results], axis=0)  # [B, A]
    return out[:, None, :].astype(np.float32)


# revision 57
# speedup vs baseline: 1.0037x; 1.0037x over previous
"""Bass/Tile TRN2 kernel for the attention module:

    pre    = prev_hidden @ W1[:H] + b1                    [B, H]
    hidden = tanh(pre[:, None, :] + ann @ W1[H:])         [B, S, H]
    score  = hidden @ W2 (+ b2; softmax-invariant, drop)  [B, S]
    alpha  = softmax(score, axis=1)
    ctx    = alpha @ ann                                  [B, 1, A]

B=32, S=4096, A=H=512. Sharding: data-parallel over batch, 4 batches per
core on 8 cores. Single pass over S per batch with an unnormalized
online softmax (scores are bounded: |score| <= sum|W2|+|b2| ~ 11.4, so
exp never overflows in fp32 and no running-max is needed):

    w_s = exp(score_s);  Z = sum w_s;  ctx = (sum w_s * ann_s) / Z

Layouts: the s-dim matmul (ann @ W1a) contracts over the feature dim a,
so it needs ann with a on SBUF partitions (annT, fp8 e4m3); the context
matmul contracts over s, so it needs natural ann (annN, bf16). The
ann @ W1a matmul runs in fp8 DoubleRow mode (K=256 per MM, 0.5
cycles/row): W1a is host-prescaled x32 into e4m3's normal range and the
1/32 is folded into the tanh activation's scale. W1h (the tiny pre
matmul, on the startup critical path feeding every tanh's bias) is
likewise fp8 x32, rescaled in the PSUM->SBUF copy.

Schedule: s-chunks of 512 are processed in PAIRS so each tanh covers
1024 elements per partition (fewer Activation-engine calls). The score
is computed TRANSPOSED (s on partitions) by 1-column matmuls whose
stationary operand is the tanh tile and whose moving operand is the W2
vector, so exp directly yields softmax-weight columns in the layout the
ctx matmuls consume -- no weight transpose exists anywhere. The ctx
matmuls likewise make annN the stationary operand and stream one
w-column each (output free size 1), accumulating per-pair PSUM groups
that the otherwise-idle DVE folds into an SBUF accumulator (PSUM allows
only one open accumulation group per zero region, so kernel-lifetime
column groups are illegal). Z comes from a ones-vector contraction of
the weight columns plus a small selector matmul at the end. Within a
pair the in-order PE interleaves step2 (fp8 DR), the previous batch's
score groups, and the previous pair's ctx groups as filler, so it never
head-blocks on the Activation engine's tanh stream. Slots iterate
b-major (all four hc of one batch before the next batch) so at startup
the Activation engine streams batch 0's tanh groups while the other
batches' annT tiles are still arriving. annT DMAs are issued one pair
ahead of use; annN (first needed by the deferred ctx tail one pair
later) trails in the same queue.
"""

import os

import numpy as np
import ml_dtypes

B = 32
S = 4096
A = 512
H = 512
NCORES = 8
BL = B // NCORES  # 4 batches per core
SC = 512          # s-chunk per matmul moving operand
NSC = S // SC     # 8
NPAIR = NSC // 2  # chunk pairs per core

BF16 = ml_dtypes.bfloat16
F8 = ml_dtypes.float8_e4m3  # maps to mybir.dt.float8e4 (TRN fp8 e4m3)
W1A_SCALE = 32.0  # W1a entries ~U(+-1/32); prescale into e4m3's normal range

_BUILT = None       # (nc,) cache — Bass module is reusable across calls
LAST_RESULT = None  # last BassKernelResults, for test harness introspection


def _build_bass(loop_n=None):
    """Build the Bass module. loop_n wraps the main pair-loop in a For_i
    executed loop_n times — a timing amplifier (outputs then meaningless);
    loop_n=None builds the real single-pass kernel."""
    from contextlib import ExitStack, nullcontext

    import concourse.bass as bass
    import concourse.tile as tile
    from concourse import bacc, mybir
    from concourse.masks import make_identity

    bf16 = mybir.dt.bfloat16
    f8 = mybir.dt.float8e4
    f32 = mybir.dt.float32

    nc = bacc.Bacc()

    annT_d = nc.dram_tensor("annT", [BL, A, S], f8, kind="ExternalInput")
    annN_d = nc.dram_tensor("annN", [BL, S, A], bf16, kind="ExternalInput")
    w1a_d = nc.dram_tensor("w1a", [A, H], f8, kind="ExternalInput")
    w1h_d = nc.dram_tensor("w1h", [H, H], f8, kind="ExternalInput")
    b1_d = nc.dram_tensor("b1", [1, H], bf16, kind="ExternalInput")
    w2_d = nc.dram_tensor("w2", [H, 1], bf16, kind="ExternalInput")
    # sel[k, b] = 1 if k % BL == b: partition-group selector for the Z
    # reduction (sums the per-(sub,st) column partials of batch b)
    sel_d = nc.dram_tensor("sel", [32, BL], f32, kind="ExternalInput")
    pvt_d = nc.dram_tensor("pvt", [H, BL], bf16, kind="ExternalInput")
    out_d = nc.dram_tensor("out", [BL, A], f32, kind="ExternalOutput")

    with tile.TileContext(nc) as tc, ExitStack() as ctx:
        singles = ctx.enter_context(tc.tile_pool(name="singles", bufs=1))
        annt_pool = ctx.enter_context(tc.tile_pool(name="annt", bufs=2))
        annn_pool = ctx.enter_context(tc.tile_pool(name="annn", bufs=2))
        th_pool = ctx.enter_context(tc.tile_pool(name="thp", bufs=2))
        w_pool = ctx.enter_context(tc.tile_pool(name="wp", bufs=2))
        psum_th = ctx.enter_context(
            tc.tile_pool(name="psumth", bufs=1, space="PSUM")
        )
        psum_sc = ctx.enter_context(
            tc.tile_pool(name="psumsc", bufs=1, space="PSUM")
        )
        psum_z = ctx.enter_context(
            tc.tile_pool(name="psumz", bufs=1, space="PSUM")
        )
        psum1 = ctx.enter_context(
            tc.tile_pool(name="psum1", bufs=1, space="PSUM")
        )

        # ---- weights / constants into SBUF ----
        # DMA queue order matters: w1h/pvt/b1 first so the pre matmul (the
        # Activation engine's gating dependency for every tanh) lands as
        # early as possible; then w1a + the first pair's annT for the PE's
        # first step2 slots.
        w1h_sb = singles.tile([128, 4, H], f8)  # (hin%128, hin//128, h)
        nc.sync.dma_start(
            out=w1h_sb, in_=w1h_d[:, :].rearrange("(kc p) h -> p kc h", p=128)
        )
        pvt_sb = singles.tile([128, 4, BL], bf16)  # (hin%128, hin//128, b)
        nc.sync.dma_start(
            out=pvt_sb, in_=pvt_d[:, :].rearrange("(kc p) b -> p kc b", p=128)
        )
        b1_sb = singles.tile([1, H], bf16)
        nc.sync.dma_start(out=b1_sb, in_=b1_d[:, :])

        w1a_sb = singles.tile([128, 4, H], f8)  # (a%128, a//128, h)
        nc.sync.dma_start(
            out=w1a_sb, in_=w1a_d[:, :].rearrange("(ac p) h -> p ac h", p=128)
        )

        at_tiles = {}  # (sp, sub, b) -> tile, DMA-issued one pair ahead

        def issue_at(sp, bs):
            for b in bs:
                for sub in range(2):
                    sc = 2 * sp + sub
                    t = annt_pool.tile([128, 4, SC], f8, tag=f"at{sub}{b}")
                    nc.sync.dma_start(
                        out=t,
                        in_=annT_d[b, :, sc * SC:(sc + 1) * SC].rearrange(
                            "(ac p) s -> p ac s", p=128
                        ),
                    )
                    at_tiles[(sp, sub, b)] = t

        issue_at(0, (0, 1))
        issue_at(0, (2, 3))

        w2_sb = singles.tile([128, 4, 1], bf16)  # (h%128, h//128, 1)
        nc.sync.dma_start(
            out=w2_sb, in_=w2_d[:, :].rearrange("(hc p) r -> p hc r", p=128)
        )
        sel_sb = singles.tile([32, BL], f32)
        nc.sync.dma_start(out=sel_sb, in_=sel_d[:, :])

        ones_sb = singles.tile([1, BL], bf16)
        nc.vector.memset(ones_sb, 1.0)
        onesc_sb = singles.tile([128, 1], bf16)
        nc.vector.memset(onesc_sb, 1.0)
        onesr_sb = singles.tile([1, 128], f32)
        nc.vector.memset(onesr_sb, 1.0)
        ones32_sb = singles.tile([32, 1], f32)
        nc.vector.memset(ones32_sb, 1.0)
        rs_sb = singles.tile([128, 1], f32)
        nc.vector.memset(rs_sb, 1.0 / W1A_SCALE)
        identf = singles.tile([128, 128], f32)
        make_identity(nc, identf)

        pre_sb = singles.tile([128, 4, BL], f32)

        def emit_pre():
            # pre2T[h, b] = (prev @ W1h).T + b1 broadcast.
            pre_ps = psum_sc.tile([128, 32], f32, tag="score")
            for hc in range(4):
                for kc in range(4):
                    nc.tensor.matmul(
                        pre_ps[:, hc * BL:(hc + 1) * BL],
                        lhsT=w1h_sb[:, kc, hc * 128:(hc + 1) * 128],
                        rhs=pvt_sb[:, kc, :],
                        start=(kc == 0),
                        stop=False,
                    )
                # b1 contribution: rank-1 with ones row (K=1)
                nc.tensor.matmul(
                    pre_ps[:, hc * BL:(hc + 1) * BL],
                    lhsT=b1_sb[:, hc * 128:(hc + 1) * 128],
                    rhs=ones_sb[:, :],
                    start=False,
                    stop=True,
                )
            nc.vector.tensor_scalar_mul(
                pre_sb,
                pre_ps[:, 0:4 * BL].rearrange("p (hc b) -> p hc b", b=BL),
                rs_sb,
            )

        ctx_acc = singles.tile([128, BL, 4], f32)  # (a%128, b, ac)
        nc.vector.memset(ctx_acc, 0.0)
        zp_acc = singles.tile([32, 1], f32)  # per-(sub,st,b) Z partials
        nc.vector.memset(zp_acc, 0.0)

        outer = (
            tc.For_i(0, loop_n, 1) if loop_n is not None else nullcontext()
        )
        with outer:
            _main_body(
                nc, tc, mybir,
                annN_d, w1a_sb, w2_sb, pre_sb,
                at_tiles, issue_at, emit_pre,
                annn_pool, th_pool, w_pool, psum_th, psum_sc, psum_z,
                psum1, onesc_sb, ctx_acc, zp_acc,
                amplified=loop_n is not None,
            )

        # ---- normalize and store ----
        # Z per batch: group-sum the [32,1] column partials via the selector
        # matmul, reciprocal, transpose to a row, broadcast over partitions,
        # then scale ctx and store.
        zpsel_sb = singles.tile([32, BL], f32)
        nc.vector.tensor_scalar_mul(zpsel_sb, sel_sb[:, :], zp_acc[:, 0:1])
        zbt_ps = psum_sc.tile([1, BL], f32, tag="score")
        nc.tensor.matmul(
            zbt_ps[:, :], lhsT=ones32_sb[:, :], rhs=zpsel_sb[:, :],
            start=True, stop=True,
        )
        zrt_sb = singles.tile([1, BL], f32)
        nc.vector.reciprocal(out=zrt_sb, in_=zbt_ps)
        bc_ps = psum_sc.tile([128, BL], f32, tag="score")
        nc.tensor.matmul(
            bc_ps[:, :], lhsT=onesr_sb[:, :], rhs=zrt_sb[:, :],
            start=True, stop=True,
        )
        out_sb = singles.tile([128, BL, 4], f32)
        for b in range(BL):
            nc.vector.tensor_scalar_mul(
                out_sb[:, b, :], ctx_acc[:, b, :], bc_ps[:, b:b + 1]
            )
        outt_ps = psum_sc.tile([4 * BL, 128], f32, tag="score")
        nc.tensor.transpose(
            outt_ps[:, :],
            out_sb[:, :, :].rearrange("p b ac -> p (b ac)"),
            identf[:, :],
        )
        outt_sb = singles.tile([4 * BL, 128], f32)
        nc.vector.tensor_copy(out=outt_sb, in_=outt_ps[:, :])
        nc.sync.dma_start(
            out=out_d[:, :].rearrange("b (ac p) -> (b ac) p", p=128),
            in_=outt_sb,
        )

    nc.finalize()
    return nc


def _main_body(
    nc, tc, mybir,
    annN_d, w1a_sb, w2_sb, pre_sb,
    at_tiles, issue_at, emit_pre,
    annn_pool, th_pool, w_pool, psum_th, psum_sc, psum_z,
    psum1, onesc_sb, ctx_acc, zp_acc,
    amplified=False,
):
    bf16 = mybir.dt.bfloat16
    f8 = mybir.dt.float8e4
    f32 = mybir.dt.float32
    Tanh = mybir.ActivationFunctionType.Tanh
    Exp = mybir.ActivationFunctionType.Exp
    DR = mybir.MatmulPerfMode.DoubleRow

    # Score is computed TRANSPOSED (s on partitions): scoreT[s, col] with
    # col = sub*16 + st*4 + b, via tiny 1-column matmuls whose STATIONARY
    # operand is the th tile (lhsT) and whose moving operand is the W2
    # vector. exp then produces the softmax weights already in the layout
    # the ctx matmuls need as moving columns, so no w transpose exists at
    # all. The ctx matmuls likewise make annN the stationary operand and
    # stream one w column (out free size 1). The ctx MMs of pair sp-1 are
    # deferred into iteration sp and spread through its slots as filler.
    emit_pre()
    pend = None
    for sp in range(NPAIR + 1):
        if sp < NPAIR:
            if amplified:
                if sp > 0:
                    issue_at(sp, range(BL))  # no prefetch under For_i
            elif sp + 1 < NPAIR:
                issue_at(sp + 1, range(BL))  # prefetch next pair's annT
            an_tiles = []  # an_tiles[b][sub]
            for b in range(BL):
                subs = []
                for sub in range(2):
                    sc = 2 * sp + sub
                    t = annn_pool.tile([128, 4, A], bf16, tag=f"an{sub}{b}")
                    nc.sync.dma_start(
                        out=t,
                        in_=annN_d[b, sc * SC:(sc + 1) * SC, :].rearrange(
                            "(sb p) a -> p sb a", p=128
                        ),
                    )
                    subs.append(t)
                an_tiles.append(subs)

            th_tiles = [
                th_pool.tile(
                    [128, 4, 2, SC], bf16, tag=f"th{b}", name=f"th{b}"
                )
                for b in range(BL)
            ]
            score_ps = psum_sc.tile([128, 32], f32, tag="score")

        if pend is not None:
            p_w, p_an, p_sp = pend
            # one job per (ac, b) ctx column: its 8 st MMs are emitted
            # CONSECUTIVELY as a complete per-pair accumulation group (PSUM
            # allows only one open group per zero region at a time); the
            # per-pair partial is then folded into ctx_acc by the DVE
            ctx_ps = psum1.tile([128, BL, 4], f32, tag="ctx")
            ctx_jobs = [(ac, b) for ac in range(4) for b in range(BL)]
        else:
            ctx_jobs = []

        def emit_zp():
            # Z partials: column-sums of the previous pair's softmax
            # weights (ones-vector contraction over the s partitions),
            # then folded into zp_acc by the DVE
            zpp = psum_z.tile([32, 1], f32, tag="zpp")
            nc.tensor.matmul(
                zpp[:, :],
                lhsT=p_w[:, :],
                rhs=onesc_sb[:, :],
                start=True,
                stop=True,
            )
            nc.vector.tensor_tensor(
                out=zp_acc[:, :], in0=zp_acc[:, :], in1=zpp[:, :],
                op=mybir.AluOpType.add,
            )

        def emit_ctx(n):
            for _ in range(n):
                if not ctx_jobs:
                    return
                ac, b = ctx_jobs.pop(0)
                for st in range(8):
                    col = (st // 4) * 16 + (st % 4) * BL + b
                    nc.tensor.matmul(
                        ctx_ps[:, b, ac:ac + 1],
                        lhsT=p_an[b][st // 4][
                            :, st % 4, ac * 128:(ac + 1) * 128
                        ],
                        rhs=p_w[:, col:col + 1],
                        start=(st == 0),
                        stop=(st == 7),
                    )

        def fold_ctx():
            nc.vector.tensor_tensor(
                out=ctx_acc[:, :, :], in0=ctx_acc[:, :, :],
                in1=ctx_ps[:, :, :],
                op=mybir.AluOpType.add,
            )

        if sp < NPAIR:

            def emit_score(b):
                # scoreT[s, col] = sum_hc th[:, s].T @ w2[:, hc]; th is the
                # stationary operand, out free size is 1. Each column's 4
                # MMs are consecutive (complete group before the next).
                for sub in range(2):
                    for st in range(4):
                        col = sub * 16 + st * BL + b
                        for hc in range(4):
                            nc.tensor.matmul(
                                score_ps[:, col:col + 1],
                                lhsT=th_tiles[b][
                                    :, hc, sub, st * 128:(st + 1) * 128
                                ],
                                rhs=w2_sb[:, hc, :],
                                start=(hc == 0),
                                stop=(hc == 3),
                            )

            slot = 0
            for b in range(BL):
                for hc in range(4):
                    thp = psum_th.tile([128, 2, SC], f32, tag=f"thp{hc % 2}")
                    for sub in range(2):
                        for kh in range(2):
                            nc.tensor.matmul(
                                thp[:, sub, :],
                                lhsT=w1a_sb[
                                    :, 2 * kh:2 * kh + 2,
                                    hc * 128:(hc + 1) * 128,
                                ],
                                rhs=at_tiles[(sp, sub, b)][
                                    :, 2 * kh:2 * kh + 2, :
                                ],
                                start=(kh == 0),
                                stop=(kh == 1),
                                perf_mode=DR,
                            )
                    nc.scalar.activation(
                        out=th_tiles[b][:, hc, :, :],
                        in_=thp[:, :, :],
                        func=Tanh,
                        bias=pre_sb[:, hc, b:b + 1],
                        scale=1.0 / W1A_SCALE,
                    )
                    if hc == 0 and b > 0:
                        emit_score(b - 1)
                    if pend is not None and slot == 0:
                        emit_zp()
                    if slot >= 2:
                        emit_ctx(2)
                    slot += 1
            emit_score(BL - 1)
            emit_ctx(len(ctx_jobs))
            if pend is not None:
                fold_ctx()

            # one exp over all 32 scoreT columns -> softmax weights with s
            # on partitions, directly consumable by the ctx matmuls
            w_sb = w_pool.tile([128, 32], bf16, tag="w")
            nc.scalar.activation(
                out=w_sb, in_=score_ps[:, :], func=Exp,
            )
            pend = (w_sb, an_tiles, sp)
        else:
            # drain: Z partials + remaining ctx MMs of the last pair
            emit_zp()
            emit_ctx(len(ctx_jobs))
            fold_ctx()
            pend = None


def _make_in_maps(prev_hidden_state, annotations, W1, b1, W2):
    prev_hidden_state = np.asarray(prev_hidden_state, dtype=np.float32)
    annotations = np.asarray(annotations, dtype=np.float32)
    W1 = np.asarray(W1, dtype=np.float32)
    b1 = np.asarray(b1, dtype=np.float32)
    W2 = np.asarray(W2, dtype=np.float32)

    annN = annotations.astype(BF16)
    annT = np.ascontiguousarray(annotations.transpose(0, 2, 1)).astype(F8)
    w1h = np.ascontiguousarray(W1[:H] * W1A_SCALE).astype(F8)
    w1a = np.ascontiguousarray(W1[H:] * W1A_SCALE).astype(F8)
    b1r = (b1 * W1A_SCALE).reshape(1, H).astype(BF16)
    w2c = np.ascontiguousarray(W2.reshape(H, 1)).astype(BF16)
    sel = np.zeros((32, BL), dtype=np.float32)
    for k in range(32):
        sel[k, k % BL] = 1.0
    pvt = np.ascontiguousarray(prev_hidden_state.T).astype(BF16)  # [H, B]

    in_maps = []
    for c in range(NCORES):
        sl = slice(c * BL, (c + 1) * BL)
        in_maps.append(
            {
                "annT": np.ascontiguousarray(annT[sl]),
                "annN": np.ascontiguousarray(annN[sl]),
                "w1a": w1a,
                "w1h": w1h,
                "b1": b1r,
                "w2": w2c,
                "sel": sel,
                "pvt": np.ascontiguousarray(pvt[:, sl]),
            }
        )
    return in_maps


def kernel(prev_hidden_state, annotations, W1, b1, W2, b2, **_unused):
    global _BUILT, LAST_RESULT
    from concourse import bass_utils

    # b2 shifts every score equally; softmax is shift-invariant -> ignored.
    in_maps = _make_in_maps(prev_hidden_state, annotations, W1, b1, W2)

    if _BUILT is None:
        _BUILT = _build_bass()
    nc = _BUILT

    trace = bool(int(os.environ.get("KERNEL_TRACE", "0")))
    if not trace:
        # the NTFF trace path needs antenv.axon_hooks, absent in this
        # client -- make sure an ambient BASS_TRACE can't select it
        os.environ.setdefault("BASS_NEVER_TRACE", "1")
    res = bass_utils.run_bass_kernel_spmd(
        nc, in_maps, core_ids=list(range(NCORES)), trace=trace
    )
    LAST_RESULT = res
    out = np.concatenate([r["out"] for r in res.results], axis=0)  # [B, A]
    return out[:, None, :].astype(np.float32)
